# revision 10
# baseline (speedup 1.0000x reference)
"""Trainium2 Bass kernel v3 for nn_BitSpikeMambaModel.

embed -> bitlinear(w0) -> bitlinear(w1) -> LN -> bitlinear(head).

Sharding:
  - trunk data-parallel: 512 tokens per core (8 cores x 512 = 4096 tokens)
  - head tensor-parallel over vocab: vocab padded to 32768, each core owns a
    4096-row slice (32 out-tiles of 128) and computes it for ALL 4096 tokens,
    after an AllGather of the LN output.

Precision (validated vs fp32 reference numerically: absmax-rel ~5e-4):
  - weights stream fp32 (ternary threshold compare must be exact in fp32;
    f16-rounded weights flip quantization decisions -> 3e-2 error)
  - head abs-mean scale pass streams a separate f16 copy (scale is a mean
    over 65M values; f16 rounding error averages out, numerically verified)
  - all activations f16 (emb gather, h1, y1, LN out), PSUM accum fp32
  - output written f16, host converts to fp32

Layouts:
  - weights pre-tiled on host to [ot, p, dt, c] (p = contraction lane,
    c = out column) so each per-out-tile DMA is 128 descriptors x 8KB
    (fp32) / 4KB (f16) at full DMA bus rate.
  - scales: w0/w1 partial abs-sums AllReduced ([1,2] early); the head
    partial rides as an extra row in the X AllGather payload (no second
    AllReduce: each core sums the 8 gathered partials locally).

v3 scheduling (vs v2):
  - dummy collective at t=0 absorbs first-collective CC latency (~60us)
  - head f16 scale stream runs during the AR1 wait window on the scalar
    queue; reduces on the otherwise-idle gpsimd engine; cross-partition
    sum via a DRAM bounce (no PE dependency); head-scale AllReduce fires
    ~100us in, long before the AllGather
  - X readback DMAs on gpsimd so sync prefetches head weights during AG
  - head matmul loop orders 4 token-chunks under one stationary weight
    tile (amortizes PE LD_WEIGHTS if the legalizer elides reloads)
  - LN scalars in f16 (no Newton step) to free SBUF
"""

import numpy as np

import concourse.bass as bass
import concourse.bacc as bacc
import concourse.mybir as mybir
import concourse.tile as tile
from concourse.bass_utils import run_bass_kernel_spmd

F32 = mybir.dt.float32
F16 = mybir.dt.float16
I16 = mybir.dt.int16
AF = mybir.ActivationFunctionType
OP = mybir.AluOpType
AX = mybir.AxisListType

VOCAB = 32000
V_PAD = 32768
DIM = 2048
BATCH = 2
SEQ = 2048
NCORES = 8
TOK = BATCH * SEQ          # 4096 total tokens
T = TOK // NCORES          # 512 tokens per core (trunk)
DT = DIM // 128            # 16 d-tiles
OT_TR = DIM // 128         # 16 trunk out-tiles
OT_HD = V_PAD // 128 // NCORES  # 32 head out-tiles per core
TC = TOK // T              # 8 token chunks in head phase
SL = 2                     # scale-slice out-tiles per core (16/8)
EPS = 1e-5


class Cfg:
    def __init__(self):
        self.key = "v3"


def build(cfg: Cfg):
    nc = bacc.Bacc("TRN2", target_bir_lowering=False, debug=False,
                   num_devices=NCORES)
    grp = [list(range(NCORES))]

    # ---- DRAM I/O ----
    idx_d = nc.dram_tensor("idx", [128, T // 16], I16, kind="ExternalInput")
    embh_d = nc.dram_tensor("embh", [VOCAB, DIM], F16, kind="ExternalInput")
    w0t_d = nc.dram_tensor("w0t", [OT_TR, 128, DT, 128], F32, kind="ExternalInput")
    w1t_d = nc.dram_tensor("w1t", [OT_TR, 128, DT, 128], F32, kind="ExternalInput")
    w0sl_d = nc.dram_tensor("w0sl", [SL, 128, DT, 128], F32, kind="ExternalInput")
    w1sl_d = nc.dram_tensor("w1sl", [SL, 128, DT, 128], F32, kind="ExternalInput")
    hwt_d = nc.dram_tensor("hwt", [OT_HD, 128, DT, 128], F32, kind="ExternalInput")
    hws_d = nc.dram_tensor("hws", [OT_HD, 128, DT, 128], F16, kind="ExternalInput")
    b0_d = nc.dram_tensor("b0r", [128, OT_TR], F32, kind="ExternalInput")
    b1_d = nc.dram_tensor("b1r", [128, OT_TR], F32, kind="ExternalInput")
    gam_d = nc.dram_tensor("gamr", [128, DT], F32, kind="ExternalInput")
    bet_d = nc.dram_tensor("betr", [128, DT], F32, kind="ExternalInput")
    hb_d = nc.dram_tensor("hbr", [128, OT_HD], F32, kind="ExternalInput")
    out_d = nc.dram_tensor("out", [OT_HD * 128, TOK], F16, kind="ExternalOutput")

    with tile.TileContext(nc) as tc:
        import contextlib
        with contextlib.ExitStack() as ctx:
            cst = ctx.enter_context(tc.tile_pool(name="cst", bufs=1))
            xall_p = ctx.enter_context(tc.tile_pool(name="xall", bufs=1))
            big = ctx.enter_context(tc.tile_pool(name="big", bufs=2))
            wstream = ctx.enter_context(tc.tile_pool(name="wstream", bufs=2))
            wscp = ctx.enter_context(tc.tile_pool(name="wscp", bufs=1))
            qbuf = ctx.enter_context(tc.tile_pool(name="qbuf", bufs=2))
            mbuf = ctx.enter_context(tc.tile_pool(name="mbuf", bufs=1))
            evt = ctx.enter_context(tc.tile_pool(name="evt", bufs=2))
            osb = ctx.enter_context(tc.tile_pool(name="osb", bufs=2))
            sml = ctx.enter_context(tc.tile_pool(name="sml", bufs=1))
            scl = ctx.enter_context(tc.tile_pool(name="scl", bufs=1))
            ps_mm = ctx.enter_context(tc.tile_pool(name="ps_mm", bufs=4, space="PSUM"))
            ps_st = ctx.enter_context(tc.tile_pool(name="ps_st", bufs=1, space="PSUM"))
            drp = ctx.enter_context(tc.tile_pool(name="drp", bufs=1, space="DRAM"))

            # ---- constants ----
            ones_col = cst.tile([128, 1], F16)
            nc.any.memset(ones_col[:], 1.0)
            ones_row = cst.tile([1, 128], F32)
            nc.any.memset(ones_row[:], 1.0)
            ones_row16 = cst.tile([1, 128], F16)
            nc.any.memset(ones_row16[:], 1.0)
            eps1 = cst.tile([1, 1], F32)
            nc.any.memset(eps1[:], EPS)
            idx_sb = cst.tile([128, T // 16], I16)
            nc.sync.dma_start(idx_sb[:], idx_d.ap())
            b0s = cst.tile([128, OT_TR], F32)
            nc.sync.dma_start(b0s[:], b0_d.ap())
            b1s = cst.tile([128, OT_TR], F32)
            nc.sync.dma_start(b1s[:], b1_d.ap())
            gams = cst.tile([128, DT], F32)
            nc.sync.dma_start(gams[:], gam_d.ap())
            bets = cst.tile([128, DT], F32)
            nc.sync.dma_start(bets[:], bet_d.ap())
            hbs = cst.tile([128, OT_HD], F32)
            nc.sync.dma_start(hbs[:], hb_d.ap())

            # ---- dummy collective: absorb first-collective CC latency ----
            ar0_in = drp.tile([1, 1], F32)
            ar0_out = drp.tile([1, 1], F32, addr_space="Shared")
            nc.sync.dma_start(ar0_in[:], eps1[:])
            nc.gpsimd.collective_compute(
                "AllReduce", OP.add, replica_groups=grp,
                ins=[ar0_in[:].opt()], outs=[ar0_out[:].opt()])

            # ---- embedding gather (gpsimd SWDGE, independent queue) ----
            xt = big.tile([128, DT, T], F16, tag="big")
            nc.gpsimd.dma_gather(out_ap=xt[:], in_ap=embh_d.ap(), idxs_ap=idx_sb[:],
                                 num_idxs=T, num_idxs_reg=T, elem_size=DIM,
                                 transpose=True)

            # ---- trunk scale slices -> AllReduce #1 ([1,2]) ----
            def slice_abs_sum(view_d, name):
                acc = scl.tile([128, DT], F32, tag=f"acc{name}")
                for i in range(SL):
                    st = wstream.tile([128, DT, 128], F32, tag="ws")
                    nc.sync.dma_start(st[:], view_d.ap()[i])
                    part = scl.tile([128, DT], F32, tag=f"part{name}")
                    nc.vector.tensor_reduce(part[:], st[:], axis=AX.X, op=OP.add,
                                            apply_absolute_value=True)
                    if i == 0:
                        nc.vector.tensor_copy(acc[:], part[:])
                    else:
                        nc.vector.tensor_tensor(acc[:], acc[:], part[:], OP.add)
                p3 = sml.tile([128, 1], F32, tag=f"p3{name}")
                nc.vector.tensor_reduce(p3[:], acc[:], axis=AX.X, op=OP.add)
                p3h = sml.tile([128, 1], F16, tag=f"p3h{name}")
                nc.vector.tensor_copy(p3h[:], p3[:])
                tps = ps_st.tile([1, 1], F32, tag="pa")
                nc.tensor.matmul(tps[:], ones_col[:], p3h[:], start=True, stop=True)
                tot = sml.tile([1, 1], F32, tag=f"tot{name}")
                nc.scalar.activation(tot[:], tps[:], AF.Copy)
                return tot

            tot0 = slice_abs_sum(w0sl_d, "w0")
            tot1 = slice_abs_sum(w1sl_d, "w1")
            pack2 = sml.tile([1, 2], F32, tag="pack2")
            nc.vector.tensor_copy(pack2[:, 0:1], tot0[:])
            nc.vector.tensor_copy(pack2[:, 1:2], tot1[:])
            ar1_in = drp.tile([1, 2], F32)
            ar1_out = drp.tile([1, 2], F32, addr_space="Shared")
            nc.sync.dma_start(ar1_in[:], pack2[:])
            nc.gpsimd.collective_compute(
                "AllReduce", OP.add, replica_groups=grp,
                ins=[ar1_in[:].opt()], outs=[ar1_out[:].opt()])
            pack2g = sml.tile([1, 2], F32, tag="pack2g")
            nc.scalar.dma_start(pack2g[:], ar1_out[:])

            # scalar [1,1] -> s=[128,1] replicated * mul, h=0.5s, nh=-0.5s
            def finalize_scale(tot_ap, mul, name):
                rps = ps_st.tile([128, 1], F32, tag="pa")
                nc.tensor.matmul(rps[:], ones_row[:], tot_ap, start=True, stop=True)
                s = scl.tile([128, 1], F32, tag=f"s{name}")
                nc.scalar.activation(s[:], rps[:], AF.Copy, scale=mul)
                nc.vector.tensor_scalar(s[:], s[:], EPS, None, OP.max)
                h = scl.tile([128, 1], F32, tag=f"h{name}")
                nc.vector.tensor_scalar(h[:], s[:], 0.5, None, OP.mult)
                nh = scl.tile([128, 1], F32, tag=f"nh{name}")
                nc.vector.tensor_scalar(nh[:], h[:], -1.0, None, OP.mult)
                return s, h, nh

            s0, h0, nh0 = finalize_scale(pack2g[:, 0:1], 1.0 / (DIM * DIM), "w0")
            s1, h1, nh1 = finalize_scale(pack2g[:, 1:2], 1.0 / (DIM * DIM), "w1")

            # ---- streamed bitlinear layer; token chunks grouped under one
            #      stationary weight tile (PE weight-load amortization) ----
            def bitlinear(wtile_d, n_ot, h_ap, nh_ap, rhs, n_tc, consume, tcg=4):
                for ot in range(n_ot):
                    wt = wstream.tile([128, DT, 128], F32, tag="ws")
                    nc.sync.dma_start(wt[:], wtile_d.ap()[ot])
                    # ternary q = 1{w > h} - 1{w < -h} (two single-op
                    # compares + subtract: ~1us/tile cheaper on DVE than the
                    # dual-op (is_lt, mult) form, keeps quant ahead of PE)
                    sgn = qbuf.tile([128, DT, 128], F16, tag="sgn")
                    nc.vector.tensor_scalar(sgn[:], wt[:], h_ap[:], None, OP.is_gt)
                    msk = mbuf.tile([128, DT, 128], F16, tag="msk")
                    nc.vector.tensor_scalar(msk[:], wt[:], nh_ap[:], None,
                                            OP.is_lt)
                    nc.vector.tensor_tensor(sgn[:], sgn[:], msk[:], OP.subtract)
                    # smaller first token-groups so head matmuls start on the
                    # first gathered chunk instead of waiting for chunk tcg-1
                    cur_tcg = min(tcg, 2) if (ot < 2 and n_tc > 1) else tcg
                    g = 0
                    while g < n_tc:
                        gtc = list(range(g, min(g + cur_tcg, n_tc)))
                        g += cur_tcg
                        cur_tcg = tcg
                        pts = [ps_mm.tile([128, T], F32, tag="ps_mm",
                                          name=f"pt{j}")
                               for j in range(len(gtc))]
                        for dt in range(DT):
                            for j, tcix in enumerate(gtc):
                                nc.tensor.matmul(
                                    pts[j][:], sgn[:, dt, :], rhs(dt, tcix),
                                    start=(dt == 0), stop=(dt == DT - 1))
                        for j, tcix in enumerate(gtc):
                            consume(ot, tcix, pts[j])

            # ---- layer 0 ----
            h1t = big.tile([128, DT, T], F16, tag="big")

            def consume_l0(ot, tcix, pt):
                nc.scalar.activation(h1t[:, ot, :], pt[:], AF.Identity,
                                     bias=b0s[:, ot:ot + 1], scale=s0[:])

            bitlinear(w0t_d, OT_TR, h0, nh0,
                      lambda dt, tcix: xt[:, dt, :], 1, consume_l0)

            # ---- layer 1 (keep f16 y1 for LN); LN stats matmuls are
            #      software-pipelined one tile behind the evictions so the
            #      PE never waits on the Act/DVE chain ----
            y1 = big.tile([128, DT, T], F16, tag="big")
            ps_s = ps_st.tile([1, T], F32, tag="ps_s")
            ps_q = ps_st.tile([1, T], F32, tag="ps_q")
            sqs = {}

            def emit_stats(ot):
                nc.tensor.matmul(ps_s[:], ones_col[:], y1[:, ot, :],
                                 start=(ot == 0), stop=(ot == DT - 1))
                nc.tensor.matmul(ps_q[:], ones_col[:], sqs.pop(ot)[:],
                                 start=(ot == 0), stop=(ot == DT - 1))

            def consume_l1(ot, tcix, pt):
                nc.scalar.activation(y1[:, ot, :], pt[:], AF.Identity,
                                     bias=b1s[:, ot:ot + 1], scale=s1[:])
                if ot >= 1:
                    emit_stats(ot - 1)
                sq = evt.tile([128, T], F16, tag="sq", bufs=2)
                nc.vector.tensor_tensor(sq[:], y1[:, ot, :], y1[:, ot, :],
                                        OP.mult)
                sqs[ot] = sq

            bitlinear(w1t_d, OT_TR, h1, nh1,
                      lambda dt, tcix: h1t[:, dt, :], 1, consume_l1)
            emit_stats(DT - 1)

            # ---- head scale pass: f16 stream + DVE reduces, right after the
            #      trunk so AllReduce #2 clears the CC cores before the
            #      AllGather fires ----
            hacc = scl.tile([128, 1], F32, tag="hacc")
            for ot in range(OT_HD):
                st = wscp.tile([128, DT, 128], F16, tag="wsc")
                nc.scalar.dma_start(st[:], hws_d.ap()[ot])
                part = scl.tile([128, 1], F32, tag="hpart")
                nc.vector.tensor_reduce(part[:], st[:], axis=AX.XYZW, op=OP.add,
                                        apply_absolute_value=True)
                if ot == 0:
                    nc.vector.tensor_copy(hacc[:], part[:])
                else:
                    nc.vector.tensor_tensor(hacc[:], hacc[:], part[:], OP.add)
            hacc16 = scl.tile([128, 1], F16, tag="hacc16")
            nc.vector.tensor_copy(hacc16[:], hacc[:])

            tpsh = ps_st.tile([1, 1], F32, tag="pa")
            nc.tensor.matmul(tpsh[:], ones_col[:], hacc16[:], start=True, stop=True)
            toth = sml.tile([1, 1], F32, tag="toth")
            nc.scalar.activation(toth[:], tpsh[:], AF.Copy)
            ar2_in = drp.tile([1, 1], F32)
            ar2_out = drp.tile([1, 1], F32, addr_space="Shared")
            nc.gpsimd.dma_start(ar2_in[:], toth[:])
            nc.gpsimd.collective_compute(
                "AllReduce", OP.add, replica_groups=grp,
                ins=[ar2_in[:].opt()], outs=[ar2_out[:].opt()])

            # LN scalars in f16 (X is f16 anyway; no Newton step needed)
            mu = sml.tile([1, T], F16, tag="mu")
            nc.scalar.activation(mu[:], ps_s[:], AF.Copy, scale=1.0 / DIM)
            ms = sml.tile([1, T], F16, tag="ms")
            nc.scalar.activation(ms[:], ps_q[:], AF.Copy, scale=1.0 / DIM)
            var = sml.tile([1, T], F16, tag="var")
            nc.vector.tensor_tensor(var[:], mu[:], mu[:], OP.mult)
            nc.vector.tensor_tensor(var[:], ms[:], var[:], OP.subtract)
            sd = sml.tile([1, T], F16, tag="sd")
            nc.scalar.activation(sd[:], var[:], AF.Sqrt, bias=eps1[:])
            rstd = sml.tile([1, T], F16, tag="ms", name="rstd")
            with nc.allow_low_precision(reason="f16 LN scalars, 2e-2 gate"):
                nc.vector.reciprocal(rstd[:], sd[:])
            murs = sml.tile([1, T], F16, tag="var", name="murs")
            nc.vector.tensor_tensor(murs[:], mu[:], rstd[:], OP.mult)
            # broadcast rstd and -mu*rstd to [128, T]
            pa = ps_st.tile([128, T], F32, tag="pa")
            nc.tensor.matmul(pa[:], ones_row16[:], rstd[:],
                             start=True, stop=True)
            a_b = cst.tile([128, T], F16)
            nc.scalar.activation(a_b[:], pa[:], AF.Copy)
            pb = ps_st.tile([128, T], F32, tag="pa")
            nc.tensor.matmul(pb[:], ones_row16[:], murs[:],
                             start=True, stop=True)
            b_b = cst.tile([128, T], F16)
            nc.scalar.activation(b_b[:], pb[:], AF.Copy, scale=-1.0)

            # ---- apply LN -> X_local f16, stream each d-tile to DRAM ----
            xg_in = drp.tile([DT * 128, T], F16)
            xg_inv = xg_in.rearrange("(dt p) t -> p dt t", p=128)
            xloc = big.tile([128, DT, T], F16, tag="big")
            for dt in range(DT):
                t1 = evt.tile([128, T], F16, tag="t1", bufs=1)
                nc.vector.tensor_tensor(t1[:], y1[:, dt, :], a_b[:], OP.mult)
                nc.vector.tensor_tensor(t1[:], t1[:], b_b[:], OP.add)
                nc.vector.tensor_scalar(xloc[:, dt, :], t1[:],
                                        gams[:, dt:dt + 1], bets[:, dt:dt + 1],
                                        OP.mult, OP.add)
                nc.sync.dma_start(xg_inv[:, dt:dt + 1, :],
                                  xloc[:, dt:dt + 1, :])

            # ---- AllGather X across cores ----
            xg_out = drp.tile([NCORES * DT * 128, T], F16, addr_space="Shared")
            nc.gpsimd.collective_compute(
                "AllGather", OP.bypass, replica_groups=grp,
                ins=[xg_in[:].opt()], outs=[xg_out[:].opt()])

            toth_g = sml.tile([1, 1], F32, tag="tothg")
            nc.sync.dma_start(toth_g[:], ar2_out[:])
            sh, hh, nhh = finalize_scale(toth_g[:], 1.0 / (DIM * VOCAB), "hd")

            # ---- read back gathered X on gpsimd (sync prefetches head w) ----
            xall = xall_p.tile([128, DT, TOK], F16)
            xg_view = xg_out.rearrange("(c dt p) t -> p dt c t", p=128, dt=DT)
            for c in range(TC):
                nc.gpsimd.dma_start(
                    xall[:, :, c * T:(c + 1) * T].rearrange(
                        "p dt (c t) -> p dt c t", c=1),
                    xg_view[:, :, c:c + 1, :])

            # ---- head ----
            def consume_head(ot, tcix, pt):
                o = osb.tile([128, T], F16, tag="osb")
                nc.scalar.activation(o[:], pt[:], AF.Identity,
                                     bias=hbs[:, ot:ot + 1], scale=sh[:])
                nc.scalar.dma_start(
                    out_d.ap()[ot * 128:(ot + 1) * 128,
                               tcix * T:(tcix + 1) * T], o[:])

            bitlinear(hwt_d, OT_HD, hh, nhh,
                      lambda dt, tcix: xall[:, dt, tcix * T:(tcix + 1) * T],
                      TC, consume_head)

    nc.compile()
    return nc


_BUILD_CACHE = {}


def _get_nc(cfg: Cfg):
    if cfg.key not in _BUILD_CACHE:
        _BUILD_CACHE[cfg.key] = build(cfg)
    return _BUILD_CACHE[cfg.key]


def _tile4(w):
    """[O, D] -> [O/128, 128(p=d lane), D/128, 128(c=o col)] contiguous."""
    O, D = w.shape
    t = w.T.reshape(D // 128, 128, O // 128, 128)   # [dt, p, ot, c]
    return np.ascontiguousarray(t.transpose(2, 1, 0, 3))


def _rearr(v, n):
    return np.ascontiguousarray(np.asarray(v, np.float32).reshape(n, 128).T)


def make_in_maps(cfg, x, emb, w0, b0, w1, b1, ln_gamma, ln_beta, head_w, head_b):
    embh = np.asarray(emb, np.float32).astype(np.float16)
    w0tl = _tile4(np.asarray(w0, np.float32))
    w1tl = _tile4(np.asarray(w1, np.float32))
    hw_pad = np.zeros((V_PAD, DIM), np.float32)
    hw_pad[:VOCAB] = np.asarray(head_w, np.float32)
    hb_pad = np.zeros((V_PAD,), np.float32)
    hb_pad[:VOCAB] = np.asarray(head_b, np.float32)
    b0r = _rearr(b0, OT_TR)
    b1r = _rearr(b1, OT_TR)
    gamr = _rearr(ln_gamma, DT)
    betr = _rearr(ln_beta, DT)

    ids = np.asarray(x).reshape(-1).astype(np.int16)
    VS = V_PAD // NCORES
    in_maps = []
    for c in range(NCORES):
        idx_arr = np.tile(ids[c * T:(c + 1) * T].reshape(T // 16, 16).T, (8, 1))
        hwt_c = _tile4(hw_pad[c * VS:(c + 1) * VS])
        in_maps.append(dict(
            idx=idx_arr, embh=embh,
            w0t=w0tl, w1t=w1tl,
            w0sl=np.ascontiguousarray(w0tl[c * SL:(c + 1) * SL]),
            w1sl=np.ascontiguousarray(w1tl[c * SL:(c + 1) * SL]),
            hwt=hwt_c, hws=hwt_c.astype(np.float16),
            b0r=b0r, b1r=b1r, gamr=gamr, betr=betr,
            hbr=_rearr(hb_pad[c * VS:(c + 1) * VS], OT_HD)))
    return in_maps


def _run(cfg: Cfg, inputs, trace=False):
    nc = _get_nc(cfg)
    in_maps = make_in_maps(cfg, **inputs)
    res = run_bass_kernel_spmd(nc, in_maps, core_ids=list(range(NCORES)),
                               trace=trace)
    outs = [res.results[c]["out"].reshape(OT_HD * 128, TOK)
            for c in range(NCORES)]
    full = np.concatenate(outs, axis=0)[:VOCAB]          # [VOCAB, TOK]
    return full, res


def kernel(**inputs) -> np.ndarray:
    cfg = Cfg()
    full, _ = _run(cfg, inputs)
    return np.ascontiguousarray(full.T).astype(np.float32).reshape(
        BATCH, SEQ, VOCAB)
